# revision 22
# baseline (speedup 1.0000x reference)
"""Distributed Trainium2 kernel for GQA attention block (dense_transformer).

Sharding: DP2 over batch x sequence-parallel-4 over query rows (8 cores),
collective-free: no cross-core communication, so no core ever waits on
another (robust to launch skew and collective overheads).

  core c -> batch b = c//4, stripe j = c%4.
  Each core owns two 256-row query chunks {iA=j, iB=7-j} of its batch
  (folded pairing: causal work per core is identical), computes the FULL
  kv projections for its batch (replicated across the 4 cores of a batch
  group), q projections for its 512 rows over ALL 32 heads, attention for
  those rows, and the full [512, 4096] output-projection rows (streaming
  all of wo).

One SPMD graph for all cores: per-core variation lives in input DATA only
(xq / cosq / mask bands).  Every core processes chunk A over k-chunks 0..7
and chunk B over k-chunks 0..15 ("union" schedule); host-packed
multiplicative mask bands zero out beyond-causal entries, which then
contribute exp(0)=1 — exactly the reference's multiplicative-mask softmax
semantics.  Chunk A's k-chunks 8..15 are always fully masked on every core
and are skipped: their exp(0)=1 contributions are added analytically
(suffix V sums + key count).

Per-core pipeline (bf16 matmuls, f32 accumulation):
  phase A1: q^T projections + fused RoPE for the 2 owned chunks (xq input)
  phase A2: kv^T projections + RoPE over full seq (streamed x^T, two
            passes over kv-head halves so weights fit in SBUF), suffix V
  phase B:  per head (32): scores^T = K^T-chunk @ q, masked exp via ACT
            (2 k-chunks per exp), attnV + ones-rowsum matmuls, epilogue
  phase C:  out rows = attn(all heads) @ wo (wo streamed per column tile)
"""

import os
import sys
import time

sys.path.insert(0, "/opt/trn_rl_repo")

import numpy as np
import ml_dtypes

import concourse.bass as bass
import concourse.mybir as mybir
import concourse.tile as tile
from concourse import bacc
from concourse.bass_utils import run_bass_kernel_spmd

BF16 = mybir.dt.bfloat16
F32 = mybir.dt.float32

B, S, D = 2, 2048, 4096
H, KVH, HD = 32, 8, 128
HALF = HD // 2
N_CORES = 8
SPG = 4  # cores per batch (sequence-parallel group)
QC = 256  # query chunk width; core j owns chunks {j, 7-j} of 8
KB = 128  # key chunk width
NKB = S // KB  # 16
KO = D // 128  # 32 contraction chunks
HC = 256  # x^T streaming half-chunk width (phase A2)
NHC = S // HC  # 8
SCALE = 1.0 / float(np.sqrt(HD))

LAST_EXEC_NS = None
_CACHE = {}


def _build(mask_mode):
    """mask_mode: 'tril' (causal specialization), 'ones' (no mask), 'full'."""
    EXT_A = NKB // 2 if mask_mode == "tril" else NKB  # k-chunks for chunk A
    EXT_B = NKB
    need_mask = mask_mode != "ones"
    if mask_mode == "tril":
        NBAND = EXT_A + EXT_B // 2  # A bands 0..7, B bands for kc 8..15
    elif mask_mode == "full":
        NBAND = EXT_A + EXT_B  # all bands
    else:
        NBAND = 0

    nc = bacc.Bacc("TRN2", target_bir_lowering=False, debug=False, num_devices=N_CORES)

    xT = nc.declare_dram_parameter("xT", [D, S], BF16, isOutput=False)
    xq = nc.declare_dram_parameter("xq", [D, 2 * QC], BF16, isOutput=False)
    wq = nc.declare_dram_parameter("wq", [D, H * HD], BF16, isOutput=False)
    wk = nc.declare_dram_parameter("wk", [D, KVH * HD], BF16, isOutput=False)
    wv = nc.declare_dram_parameter("wv", [D, KVH * HD], BF16, isOutput=False)
    wo = nc.declare_dram_parameter("wo", [D, D], BF16, isOutput=False)
    cos2 = nc.declare_dram_parameter("cos2", [128, S], F32, isOutput=False)
    sin2 = nc.declare_dram_parameter("sin2", [128, S], F32, isOutput=False)
    cosq = nc.declare_dram_parameter("cosq", [128, 2 * QC], F32, isOutput=False)
    sinq = nc.declare_dram_parameter("sinq", [128, 2 * QC], F32, isOutput=False)
    if need_mask:
        maskb = nc.declare_dram_parameter("maskb", [128, NBAND * QC], BF16,
                                          isOutput=False)
    out = nc.declare_dram_parameter("out", [2 * QC, D], F32, isOutput=True)

    wq_r = wq.ap().rearrange("(ko p) n -> p ko n", p=128)
    wk_r = wk.ap().rearrange("(ko p) n -> p ko n", p=128)
    wv_r = wv.ap().rearrange("(ko p) n -> p ko n", p=128)
    wo_r = wo.ap().rearrange("(ko p) n -> p ko n", p=128)
    xT_r = xT.ap().rearrange("(ko p) s -> p ko s", p=128)
    xq_r = xq.ap().rearrange("(ko p) s -> p ko s", p=128)

    with tile.TileContext(nc) as tc:
        with tc.tile_pool(name="persist", bufs=1) as persist:
            qT = persist.tile([128, H, 2 * QC], BF16)
            kT = persist.tile([128, KVH, S], BF16)
            V = persist.tile([128, NKB, KVH * HD], BF16)  # [seq128, kc, kvh*hd]
            ones_sb = persist.tile([128, 128], BF16)
            nc.vector.memset(ones_sb[:], 1.0)
            if mask_mode == "tril":
                corrA = persist.tile([128, KVH], F32)  # suffix V sums kc>=8

            # ---------- phase A1: q projections + RoPE -------------------
            with (
                tc.tile_pool(name="pa1_c", bufs=1) as pa1_c,
                tc.tile_pool(name="pa1_x", bufs=1) as pa1_x,
                tc.tile_pool(name="pa1_w", bufs=2) as pa1_w,
                tc.tile_pool(name="pa1_t", bufs=2) as pa1_t,
                tc.tile_pool(name="pa1_ps", bufs=4, space="PSUM") as pa1_ps,
            ):
                cosq_sb = pa1_c.tile([128, 2 * QC], F32, tag="cq")
                sinq_sb = pa1_c.tile([128, 2 * QC], F32, tag="sq")
                xq_sb = pa1_x.tile([128, KO, 2 * QC], BF16, tag="xq")
                # pair-0 weights first so the PE unblocks after ~1.5 MB of DMA
                wq_a0 = pa1_w.tile([128, KO, 128], BF16, tag="wqa")
                wq_b0 = pa1_w.tile([128, KO, 128], BF16, tag="wqb")
                nc.sync.dma_start(wq_a0[:], wq_r[:, :, 0:128])
                nc.sync.dma_start(xq_sb[:, 0:4, :], xq_r[:, 0:4, :])
                nc.sync.dma_start(wq_b0[:], wq_r[:, :, 128:256])
                for ko in range(4, KO, 4):
                    k4 = slice(ko, ko + 4)
                    nc.sync.dma_start(xq_sb[:, k4, :], xq_r[:, k4, :])
                    if ko == 4:
                        nc.sync.dma_start(cosq_sb[:], cosq.ap())
                        nc.sync.dma_start(sinq_sb[:], sinq.ap())

                for p in range(H // 2):
                    if p == 0:
                        wq_a, wq_b = wq_a0, wq_b0
                    else:
                        wq_a = pa1_w.tile([128, KO, 128], BF16, tag="wqa")
                        wq_b = pa1_w.tile([128, KO, 128], BF16, tag="wqb")
                        nc.sync.dma_start(wq_a[:], wq_r[:, :, p * 256 : p * 256 + 128])
                        nc.sync.dma_start(
                            wq_b[:], wq_r[:, :, p * 256 + 128 : p * 256 + 256])
                    ps_a = pa1_ps.tile([128, 2 * QC], F32, tag="ps")
                    ps_b = pa1_ps.tile([128, 2 * QC], F32, tag="ps")
                    for ko in range(KO):
                        st, sp = ko == 0, ko == KO - 1
                        nc.tensor.matmul(ps_a[:], lhsT=wq_a[:, ko, :],
                                         rhs=xq_sb[:, ko, :], start=st, stop=sp)
                        nc.tensor.matmul(ps_b[:], lhsT=wq_b[:, ko, :],
                                         rhs=xq_sb[:, ko, :], start=st, stop=sp)
                    # rope + de-interleave repack
                    t1 = pa1_t.tile([128, 2 * QC], F32, tag="t1")
                    t2 = pa1_t.tile([128, 2 * QC], F32, tag="t2")
                    ra = pa1_t.tile([128, 2 * QC], BF16, tag="r")
                    rb = pa1_t.tile([128, 2 * QC], BF16, tag="r", name="rb")
                    nc.vector.tensor_mul(t1[:], ps_a[:], cosq_sb[:])
                    nc.vector.tensor_mul(t2[:], ps_b[:], sinq_sb[:])
                    nc.vector.tensor_sub(ra[:], t1[:], t2[:])
                    nc.vector.tensor_mul(t1[:], ps_a[:], sinq_sb[:])
                    nc.vector.tensor_mul(t2[:], ps_b[:], cosq_sb[:])
                    nc.vector.tensor_add(rb[:], t1[:], t2[:])
                    h1, h2 = 2 * p, 2 * p + 1
                    nc.sync.dma_start(qT[0:64, h1, :], ra[0:64, :])
                    nc.sync.dma_start(qT[0:64, h2, :], ra[64:128, :])
                    nc.sync.dma_start(qT[64:128, h1, :], rb[0:64, :])
                    nc.sync.dma_start(qT[64:128, h2, :], rb[64:128, :])

            # ---------- phase A2: kv projections + RoPE (2 passes) -------
            with (
                tc.tile_pool(name="pa2_w", bufs=1) as pa2_w,
                tc.tile_pool(name="pa2_x", bufs=2) as pa2_x,
                tc.tile_pool(name="pa2_c", bufs=2) as pa2_c,
                tc.tile_pool(name="pa2_t", bufs=2) as pa2_t,
                tc.tile_pool(name="pa2_ps", bufs=6, space="PSUM") as pa2_ps,
                tc.tile_pool(name="pa2_psv", bufs=2, space="PSUM") as pa2_psv,
            ):
                for ph in range(2):  # kv-head halves
                    wcol = slice(ph * 512, ph * 512 + 512)
                    wk_h = pa2_w.tile([128, KO, 512], BF16, tag="wk", name=f"wk{ph}")
                    wv_h = pa2_w.tile([128, KO, 512], BF16, tag="wv", name=f"wv{ph}")
                    xs_first = None
                    if ph == 0:
                        # interleave so the first k-proj chain unblocks after
                        # ~1.5 MB: wk ko0-7, then hc0's x, then the rest
                        nc.sync.dma_start(wk_h[:, 0:8, :], wk_r[:, 0:8, wcol])
                        xs_first = pa2_x.tile([128, KO, HC], BF16, tag="x")
                        for ko in range(0, KO, 4):
                            k4 = slice(ko, ko + 4)
                            nc.sync.dma_start(xs_first[:, k4, :], xT_r[:, k4, 0:HC])
                        for ko in range(8, KO, 8):
                            k8 = slice(ko, ko + 8)
                            nc.sync.dma_start(wk_h[:, k8, :], wk_r[:, k8, wcol])
                        for ko in range(0, KO, 8):
                            k8 = slice(ko, ko + 8)
                            nc.sync.dma_start(wv_h[:, k8, :], wv_r[:, k8, wcol])
                    else:
                        for ko in range(0, KO, 8):
                            k8 = slice(ko, ko + 8)
                            nc.sync.dma_start(wk_h[:, k8, :], wk_r[:, k8, wcol])
                            nc.sync.dma_start(wv_h[:, k8, :], wv_r[:, k8, wcol])
                    for hc in range(NHC):
                        sl = slice(hc * HC, (hc + 1) * HC)
                        if hc == 0 and xs_first is not None:
                            xs = xs_first
                        else:
                            xs = pa2_x.tile([128, KO, HC], BF16, tag="x")
                            for ko in range(0, KO, 4):
                                k4 = slice(ko, ko + 4)
                                nc.sync.dma_start(xs[:, k4, :], xT_r[:, k4, sl])
                        cos_c = pa2_c.tile([128, HC], F32, tag="cos")
                        sin_c = pa2_c.tile([128, HC], F32, tag="sin")
                        nc.sync.dma_start(cos_c[:], cos2.ap()[:, sl])
                        nc.sync.dma_start(sin_c[:], sin2.ap()[:, sl])
                        # 2 k-pairs in this half
                        for p in range(2):
                            gp = ph * 2 + p  # global pair id
                            ps_a = pa2_ps.tile([128, HC], F32, tag="ps")
                            ps_b = pa2_ps.tile([128, HC], F32, tag="ps")
                            for ko in range(KO):
                                st, sp = ko == 0, ko == KO - 1
                                nc.tensor.matmul(
                                    ps_a[:], lhsT=wk_h[:, ko, p * 256 : p * 256 + 128],
                                    rhs=xs[:, ko, :], start=st, stop=sp)
                                nc.tensor.matmul(
                                    ps_b[:],
                                    lhsT=wk_h[:, ko, p * 256 + 128 : p * 256 + 256],
                                    rhs=xs[:, ko, :], start=st, stop=sp)
                            t1 = pa2_t.tile([128, HC], F32, tag="t1")
                            t2 = pa2_t.tile([128, HC], F32, tag="t2")
                            ra = pa2_t.tile([128, HC], BF16, tag="r")
                            rb = pa2_t.tile([128, HC], BF16, tag="r", name="rb")
                            nc.vector.tensor_mul(t1[:], ps_a[:], cos_c[:])
                            nc.vector.tensor_mul(t2[:], ps_b[:], sin_c[:])
                            nc.vector.tensor_sub(ra[:], t1[:], t2[:])
                            nc.vector.tensor_mul(t1[:], ps_a[:], sin_c[:])
                            nc.vector.tensor_mul(t2[:], ps_b[:], cos_c[:])
                            nc.vector.tensor_add(rb[:], t1[:], t2[:])
                            h1, h2 = 2 * gp, 2 * gp + 1
                            nc.sync.dma_start(kT[0:64, h1, sl], ra[0:64, :])
                            nc.sync.dma_start(kT[0:64, h2, sl], ra[64:128, :])
                            nc.sync.dma_start(kT[64:128, h1, sl], rb[0:64, :])
                            nc.sync.dma_start(kT[64:128, h2, sl], rb[64:128, :])
                        # V natural [seq, 512 cols of this half]
                        for sb in range(HC // 128):
                            kc = hc * (HC // 128) + sb
                            ps_v = pa2_psv.tile([128, 512], F32, tag="psv")
                            for ko in range(KO):
                                nc.tensor.matmul(
                                    ps_v[:], lhsT=xs[:, ko, sb * 128 : sb * 128 + 128],
                                    rhs=wv_h[:, ko, :],
                                    start=(ko == 0), stop=(ko == KO - 1))
                            nc.scalar.copy(
                                V[:, kc, ph * 512 : ph * 512 + 512], ps_v[:])

            # suffix V sums for chunk A's skipped k-chunks (kc >= 8)
            if mask_mode == "tril":
                with tc.tile_pool(name="pcorr", bufs=2, space="PSUM") as pcorr:
                    for hk in range(KVH):
                        psc = pcorr.tile([128, 1], F32, tag="psc")
                        kcs = list(range(EXT_A, NKB))
                        for i, kc in enumerate(kcs):
                            nc.tensor.matmul(
                                psc[:], lhsT=V[:, kc, hk * HD : (hk + 1) * HD],
                                rhs=ones_sb[:, 0:1],
                                start=(i == 0), stop=(i == len(kcs) - 1))
                        nc.scalar.copy(corrA[:, hk : hk + 1], psc[:])

            # ================= phases B + C ==============================
            with tc.tile_pool(name="pbc", bufs=1) as pbc:
                outT = pbc.tile([128, H, 2 * QC], BF16)
                if need_mask:
                    mask_sb = pbc.tile([128, NBAND, QC], BF16)
                    nc.sync.dma_start(
                        mask_sb[:],
                        maskb.ap().rearrange("p (c q) -> p c q", q=QC))

                # phase-C weight pool opens here so the first wo tile's DMA
                # overlaps phase B instead of stalling phase C's start
                pcw_ctx = tc.tile_pool(name="pc_w", bufs=2)
                pc_w = pcw_ctx.__enter__()
                NT = 256  # phase-C output column tile
                wo_first = pc_w.tile([128, KO, NT], BF16, tag="won")
                for ko in range(0, KO, 8):
                    nc.sync.dma_start(wo_first[:, ko : ko + 8, :],
                                      wo_r[:, ko : ko + 8, 0:NT])
                with (
                    tc.tile_pool(name="pb_e", bufs=6) as pb_e,
                    tc.tile_pool(name="pb_s", bufs=4) as pb_s,
                    tc.tile_pool(name="pb_ps_s", bufs=4, space="PSUM") as pb_ps_s,
                    tc.tile_pool(name="pb_ps_o", bufs=2, space="PSUM") as pb_ps_o,
                    tc.tile_pool(name="pb_ps_r", bufs=2, space="PSUM") as pb_ps_r,
                ):
                    for h in range(H):
                        hk = h // (H // KVH)
                        for ci, ext, qoff in ((0, EXT_A, 0), (1, EXT_B, QC)):
                            q_rhs = qT[:, h, qoff : qoff + QC]
                            ps_o = pb_ps_o.tile([128, QC], F32, tag="ps_o")
                            ps_r = pb_ps_r.tile([128, QC], F32, tag="ps_r")
                            npair = ext // 2
                            for pp in range(npair):
                                kc0, kc1 = 2 * pp, 2 * pp + 1
                                ps_s = pb_ps_s.tile([128, 2 * QC], F32, tag="ps_s")
                                nc.tensor.matmul(
                                    ps_s[:, 0:QC],
                                    lhsT=kT[:, hk, kc0 * KB : (kc0 + 1) * KB],
                                    rhs=q_rhs, start=True, stop=False)
                                nc.tensor.matmul(
                                    ps_s[:, QC : 2 * QC],
                                    lhsT=kT[:, hk, kc1 * KB : (kc1 + 1) * KB],
                                    rhs=q_rhs, start=False, stop=True)
                                # mask bands: chunk A always (tril/full);
                                # chunk B: kc >= 8 (tril) or all (full)
                                band = None
                                if need_mask:
                                    if ci == 0:
                                        band = kc0
                                    elif mask_mode == "full":
                                        band = EXT_A + kc0
                                    elif kc0 >= EXT_B // 2:
                                        band = EXT_A + (kc0 - EXT_B // 2)
                                if band is not None:
                                    nc.vector.tensor_mul(
                                        ps_s[:], ps_s[:],
                                        mask_sb[:, band : band + 2, :]
                                        .rearrange("p c q -> p (c q)"))
                                ex = pb_e.tile([128, 2 * QC], BF16, tag="ex")
                                nc.scalar.activation(
                                    ex[:], ps_s[:],
                                    mybir.ActivationFunctionType.Exp, scale=SCALE)
                                st, sp = pp == 0, pp == npair - 1
                                nc.tensor.matmul(
                                    ps_o[:], lhsT=V[:, kc0, hk * HD : (hk + 1) * HD],
                                    rhs=ex[:, 0:QC], start=st, stop=False)
                                nc.tensor.matmul(
                                    ps_o[:], lhsT=V[:, kc1, hk * HD : (hk + 1) * HD],
                                    rhs=ex[:, QC : 2 * QC], start=False, stop=sp)
                                nc.tensor.matmul(
                                    ps_r[:], lhsT=ones_sb[:], rhs=ex[:, 0:QC],
                                    start=st, stop=False)
                                nc.tensor.matmul(
                                    ps_r[:], lhsT=ones_sb[:], rhs=ex[:, QC : 2 * QC],
                                    start=False, stop=sp)
                            # epilogue
                            dst = outT[:, h, qoff : qoff + QC]
                            rcp = pb_s.tile([128, QC], F32, tag="rcp")
                            if ci == 0 and mask_mode == "tril":
                                rs = pb_s.tile([128, QC], F32, tag="rs")
                                nc.vector.tensor_scalar_add(
                                    rs[:], ps_r[:], float((NKB - EXT_A) * KB))
                                nc.vector.reciprocal_approx_fast(rcp[:], rs[:])
                                nc.vector.scalar_tensor_tensor(
                                    dst, ps_o[:], corrA[:, hk : hk + 1], rcp[:],
                                    op0=mybir.AluOpType.add,
                                    op1=mybir.AluOpType.mult)
                            else:
                                nc.vector.reciprocal_approx_fast(rcp[:], ps_r[:])
                                nc.vector.tensor_mul(dst, ps_o[:], rcp[:])

                # ---------------- phase C (wo matmul) --------------------
                with (
                    tc.tile_pool(name="pc_o", bufs=2) as pc_o,
                    tc.tile_pool(name="pc_ps", bufs=4, space="PSUM") as pc_ps,
                ):
                    for n in range(D // NT):
                        nsl = slice(n * NT, (n + 1) * NT)
                        if n == 0:
                            wo_n = wo_first
                        else:
                            wo_n = pc_w.tile([128, KO, NT], BF16, tag="won")
                            for ko in range(0, KO, 8):
                                k8 = slice(ko, ko + 8)
                                nc.sync.dma_start(wo_n[:, k8, :], wo_r[:, k8, nsl])
                        for m in range(2 * QC // 128):
                            ps = pc_ps.tile([128, NT], F32, tag="pc")
                            for h in range(H):
                                nc.tensor.matmul(
                                    ps[:], lhsT=outT[:, h, m * 128 : (m + 1) * 128],
                                    rhs=wo_n[:, h, :],
                                    start=h == 0, stop=h == H - 1)
                            ob = pc_o.tile([128, NT], F32, tag="ob")
                            nc.scalar.copy(ob[:], ps[:])
                            nc.sync.dma_start(
                                out.ap()[m * 128 : (m + 1) * 128, nsl], ob[:])
                pcw_ctx.__exit__(None, None, None)

    nc.compile()
    return nc


def _prep_inputs(x, freqs_cos, freqs_sin, mask, wq, wk, wv, wo, mask_mode):
    bf16 = ml_dtypes.bfloat16
    EXT_A = NKB // 2 if mask_mode == "tril" else NKB
    wq4 = wq.reshape(D, H, HD)
    wk4 = wk.reshape(D, KVH, HD)

    cosT = np.ascontiguousarray(freqs_cos.T).astype(np.float32)  # [64, S]
    sinT = np.ascontiguousarray(freqs_sin.T).astype(np.float32)
    cos2 = np.concatenate([cosT, cosT], axis=0)  # [128, S]
    sin2 = np.concatenate([sinT, sinT], axis=0)
    maskT_np = None
    if mask_mode != "ones":
        maskT_np = np.ascontiguousarray(mask.T).astype(np.float32)

    def pack_pairs(w4, nheads):
        cols = []
        for p in range(nheads // 2):
            h1, h2 = 2 * p, 2 * p + 1
            cols.append(w4[:, h1, 0::2])
            cols.append(w4[:, h2, 0::2])
            cols.append(w4[:, h1, 1::2])
            cols.append(w4[:, h2, 1::2])
        return np.ascontiguousarray(np.concatenate(cols, axis=1)).astype(bf16)

    wq_p = pack_pairs(wq4, H)  # [D, 4096]
    wk_p = pack_pairs(wk4, KVH)  # [D, 1024]
    wv_n = np.ascontiguousarray(wv).astype(bf16)  # [D, 1024]
    wo_n = np.ascontiguousarray(wo).astype(bf16)  # [D, D]

    in_maps = []
    for c in range(N_CORES):
        b, j = divmod(c, SPG)
        iA, iB = j, (S // QC - 1) - j
        xTb = np.ascontiguousarray(x[b].T).astype(bf16)  # [D, S]
        csl = np.r_[iA * QC : (iA + 1) * QC, iB * QC : (iB + 1) * QC]
        m = {
            "xT": xTb,
            "xq": np.ascontiguousarray(xTb[:, csl]),
            "wq": wq_p, "wk": wk_p, "wv": wv_n, "wo": wo_n,
            "cos2": cos2, "sin2": sin2,
            "cosq": np.ascontiguousarray(cos2[:, csl]),
            "sinq": np.ascontiguousarray(sin2[:, csl]),
        }
        if maskT_np is not None:
            bands = []
            for kc in range(EXT_A):  # chunk A bands
                bands.append(maskT_np[kc * KB : (kc + 1) * KB,
                                      iA * QC : (iA + 1) * QC])
            b_lo = 0 if mask_mode == "full" else NKB // 2
            for kc in range(b_lo, NKB):  # chunk B bands
                bands.append(maskT_np[kc * KB : (kc + 1) * KB,
                                      iB * QC : (iB + 1) * QC])
            m["maskb"] = np.ascontiguousarray(
                np.concatenate(bands, axis=1)).astype(bf16)
        in_maps.append(m)
    return in_maps


def _bench_chained(nc, in_maps, iters=256):
    """Per-iteration wall time of back-to-back SPMD executions.

    Outputs of iteration i are donated as the (dead) output buffers of
    iteration i+1, so no host->device transfers happen inside the timed
    loop — the measurement is the device-side execution rate.
    """
    import jax
    from jax.sharding import Mesh, PartitionSpec, NamedSharding
    from jax.experimental.shard_map import shard_map
    from concourse import bass2jax
    from concourse.bass2jax import _bass_exec_p, partition_id_tensor

    bass2jax.install_neuronx_cc_hook()
    partition_name = nc.partition_id_tensor.name if nc.partition_id_tensor else None
    in_names, out_names, out_avals, zero_outs = [], [], [], []
    for alloc in nc.m.functions[0].allocations:
        if not isinstance(alloc, mybir.MemoryLocationSet):
            continue
        name = alloc.memorylocations[0].name
        if alloc.kind == "ExternalInput":
            if name != partition_name:
                in_names.append(name)
        elif alloc.kind == "ExternalOutput":
            out_names.append(name)
            shape = tuple(alloc.tensor_shape)
            dtype = mybir.dt.np(alloc.dtype)
            out_avals.append(jax.core.ShapedArray(shape, dtype))
            zero_outs.append(np.zeros(shape, dtype))
    n_params = len(in_names)
    n_outs = len(out_avals)
    all_in_names = list(in_names) + out_names
    if partition_name is not None:
        all_in_names.append(partition_name)

    def _body(*args):
        operands = list(args)
        if partition_name is not None:
            operands.append(partition_id_tensor())
        return tuple(_bass_exec_p.bind(
            *operands, out_avals=tuple(out_avals), in_names=tuple(all_in_names),
            out_names=tuple(out_names), lowering_input_output_aliases=(),
            sim_require_finite=False, sim_require_nnan=False, nc=nc))

    devices = jax.devices()[:N_CORES]
    mesh = Mesh(np.asarray(devices), ("core",))
    in_specs = (PartitionSpec("core"),) * (n_params + n_outs)
    out_specs = (PartitionSpec("core"),) * n_outs
    donate = tuple(range(n_params, n_params + n_outs))
    sharded = jax.jit(
        shard_map(_body, mesh=mesh, in_specs=in_specs, out_specs=out_specs,
                  check_rep=False),
        donate_argnums=donate, keep_unused=True)

    shard = NamedSharding(mesh, PartitionSpec("core"))
    concat_in = [
        jax.device_put(
            np.concatenate([np.asarray(in_maps[c][k]) for c in range(N_CORES)],
                           axis=0),
            shard)
        for k in in_names
    ]
    outs = [
        jax.device_put(np.zeros((N_CORES * z.shape[0], *z.shape[1:]), z.dtype), shard)
        for z in zero_outs
    ]
    outs = list(sharded(*concat_in, *outs))  # warmup (compiles) + 1st exec
    jax.block_until_ready(outs)
    t0 = time.perf_counter()
    for _ in range(iters):
        outs = list(sharded(*concat_in, *outs))
    jax.block_until_ready(outs)
    t1 = time.perf_counter()
    return (t1 - t0) / iters * 1e9


def bench(nc, in_maps, iters=256):
    return _bench_chained(nc, in_maps, iters=iters)


def kernel(x, freqs_cos, freqs_sin, mask, wq, wk, wv, wo, cache_k, cache_v, start_pos):
    global LAST_EXEC_NS
    x = np.asarray(x, np.float32)
    mask = np.asarray(mask, np.float32)
    assert int(start_pos) == 0, "kernel specialized for start_pos == 0"
    assert x.shape == (B, S, D)

    tril = np.tril(np.ones((S, S), np.float32))
    if np.array_equal(mask, tril):
        mask_mode = "tril"
    elif np.all(mask == 1.0):
        mask_mode = "ones"
    else:
        mask_mode = "full"

    if mask_mode not in _CACHE:
        _CACHE[mask_mode] = _build(mask_mode)
    nc = _CACHE[mask_mode]

    in_maps = _prep_inputs(
        x, np.asarray(freqs_cos, np.float32), np.asarray(freqs_sin, np.float32),
        mask, np.asarray(wq, np.float32), np.asarray(wk, np.float32),
        np.asarray(wv, np.float32), np.asarray(wo, np.float32), mask_mode)

    trace = bool(os.environ.get("KERNEL_TRACE"))
    try:
        res = run_bass_kernel_spmd(nc, in_maps, list(range(N_CORES)), trace=trace)
    except (ImportError, ModuleNotFoundError):
        # NTFF profiling hooks unavailable in this environment
        res = run_bass_kernel_spmd(nc, in_maps, list(range(N_CORES)), trace=False)
    LAST_EXEC_NS = res.exec_time_ns

    if LAST_EXEC_NS is None and not os.environ.get("KERNEL_NO_BENCH"):
        # no HW profile available: measure steady-state per-iteration device
        # time with a chained-donation pipeline (no host transfers timed)
        try:
            LAST_EXEC_NS = int(_bench_chained(nc, in_maps, iters=1536))
        except Exception:
            LAST_EXEC_NS = None

    out = np.empty((B, S, D), np.float32)
    for c in range(N_CORES):
        b, j = divmod(c, SPG)
        iA, iB = j, (S // QC - 1) - j
        r = res.results[c]["out"]
        out[b, iA * QC : (iA + 1) * QC, :] = r[0:QC]
        out[b, iB * QC : (iB + 1) * QC, :] = r[QC : 2 * QC]
    return out
